# revision 12
# baseline (speedup 1.0000x reference)
"""MMoE (8-expert TCN mixture) Trainium2 kernel.

Data-parallel over batch: 8 cores x 64 samples, all params replicated.
All matmuls in float32r (11-bit RNE mantissa, ~full PE rate at K=128) with a
hi/lo-split gate matmul (softmax over 25600-term logits needs ~fp32 accuracy).

Self-contained: hardcodes shapes, preprocesses inputs on host (padding,
shifted-duplicate layouts, f32r pre-rounding), builds one Bass/Tile program,
runs it SPMD on 8 NeuronCores, reassembles the full [4, 512, 6400] output.
"""
import os
import sys

sys.path.insert(0, "/opt/trn_rl_repo")

import numpy as np
import ml_dtypes

import concourse.bass as bass
import concourse.tile as tile
from concourse import bacc, mybir
from concourse.bass_utils import run_bass_kernel_spmd

F32 = mybir.dt.float32
F32R = mybir.dt.float32r
BF16 = mybir.dt.bfloat16

B, F, L = 512, 64, 400
E, T = 8, 4
C1, C2, KW = 64, 16, 3
U = C2 * L
NCORES = 8
BC = B // NCORES          # samples per core
LP = 402                  # padded seq len for conv1 input (left pad 2)
HP = 404                  # padded seq len for conv2 input (left pad 4)
NLPAIR = L // 2           # 200 gate l-pair chunks
GCHUNK = 20               # l-pairs per gate stream chunk

LAST_EXEC_NS = None       # set after each kernel() call when tracing


def f32r_round(a: np.ndarray) -> np.ndarray:
    """Round fp32 to the float32r grid: RNE to 11 explicit mantissa bits."""
    u = np.ascontiguousarray(a, dtype=np.float32).view(np.uint32)
    s = u & np.uint32(0x80000000)
    m = u & np.uint32(0x7FFFFFFF)
    lsb = (m >> np.uint32(12)) & np.uint32(1)
    m2 = (m + np.uint32(0x7FF) + lsb) & np.uint32(0xFFFFF000)
    return (s | m2).view(np.float32)


def build_nc():
    nc = bacc.Bacc("TRN2", target_bir_lowering=False)

    x2h = nc.dram_tensor("x2h", [128, BC, LP], F32R, kind="ExternalInput")
    x2l = nc.dram_tensor("x2l", [128, LP, BC], BF16, kind="ExternalInput")
    x2g = nc.dram_tensor("x2g", [128, LP, BC], F32R, kind="ExternalInput")
    gkh = nc.dram_tensor("gkh", [128, NLPAIR, 32], F32R, kind="ExternalInput")
    gkhb = nc.dram_tensor("gkhb", [128, NLPAIR, 32], BF16, kind="ExternalInput")
    gkl = nc.dram_tensor("gkl", [128, NLPAIR, 32], F32R, kind="ExternalInput")
    w1a = nc.dram_tensor("w1a", [128, 4, 128], F32R, kind="ExternalInput")
    w1b = nc.dram_tensor("w1b", [128, 4, 128], F32R, kind="ExternalInput")
    w2 = nc.dram_tensor("w2", [128, 12, 32], F32R, kind="ExternalInput")
    b1 = nc.dram_tensor("b1", [128, 4], F32, kind="ExternalInput")
    ebias = nc.dram_tensor("ebias", [128, 400], F32, kind="ExternalInput")
    b2 = nc.dram_tensor("b2", [32, 4], F32, kind="ExternalInput")
    d16 = nc.dram_tensor("d16", [128, 16], F32, kind="ExternalInput")
    e8 = nc.dram_tensor("e8", [8, 128], F32, kind="ExternalInput")
    tb = nc.dram_tensor("tb", [64, 1], F32, kind="ExternalInput")
    eye = nc.dram_tensor("eye", [64, 64], F32, kind="ExternalInput")
    gbb1 = nc.dram_tensor("gbb1", [1, 32], F32, kind="ExternalInput")
    ones1 = nc.dram_tensor("ones1", [1, 64], F32, kind="ExternalInput")
    out = nc.dram_tensor("out", [T, BC, U], F32, kind="ExternalOutput")

    with tile.TileContext(nc) as tc:
        with (
            tc.tile_pool(name="const", bufs=1) as cst,
            tc.tile_pool(name="xres", bufs=1) as xres,
            tc.tile_pool(name="gstream", bufs=2) as gst,
            tc.tile_pool(name="work", bufs=1) as wk,
            tc.tile_pool(name="conv", bufs=1) as cv,
            tc.tile_pool(name="ps", bufs=1, space="PSUM") as ps,
        ):
            # ---- constants ----
            w1a_t = cst.tile([128, 4, 128], F32R)
            w1b_t = cst.tile([128, 4, 128], F32R)
            w2_t = cst.tile([128, 12, 32], F32R)
            b1_t = cst.tile([128, 4], F32)
            ebias_t = cst.tile([128, 400], F32)
            b2_t = cst.tile([32, 4], F32)
            d16_t = cst.tile([128, 16], F32)
            e8_t = cst.tile([8, 128], F32)
            tb_t = cst.tile([64, 1], F32)
            eye_t = cst.tile([64, 64], F32)
            gbb1_t = cst.tile([1, 32], F32)
            ones1_t = cst.tile([1, 64], F32)
            for tl, dr in ((w1a_t, w1a), (w1b_t, w1b), (w2_t, w2), (b1_t, b1),
                           (ebias_t, ebias), (b2_t, b2), (d16_t, d16), (e8_t, e8),
                           (tb_t, tb), (eye_t, eye), (gbb1_t, gbb1), (ones1_t, ones1)):
                nc.sync.dma_start(out=tl[:], in_=dr[:])

            # ---- resident x (f32r, tap-shifted duplicate layout) ----
            x2h_t = xres.tile([128, BC, LP], F32R)
            nc.gpsimd.dma_start(out=x2h_t[:], in_=x2h[:])

            # =========== gates phase ===========
            lgp = ps.tile([32, 64], F32, tag="lg")
            nc.tensor.matmul(lgp[:], gbb1_t[:], ones1_t[:], start=True, stop=False)
            first = False
            nmm = 3 * NLPAIR
            imm = 0
            for cs in range(0, NLPAIR, GCHUNK):
                ce = min(cs + GCHUNK, NLPAIR)
                ncols = 2 * (ce - cs)
                xlc = gst.tile([128, 2 * GCHUNK, BC], BF16, tag="xlc")
                nc.sync.dma_start(out=xlc[:, :ncols, :], in_=x2l[:, 2 * cs + 2:2 * ce + 2, :])
                use_xg = True
                if use_xg:
                    xgc = gst.tile([128, 2 * GCHUNK, BC], F32R, tag="xgc",
                                   name=f"xgc_{cs}")
                    nc.sync.dma_start(out=xgc[:, :ncols, :],
                                      in_=x2g[:, 2 * cs + 2:2 * ce + 2, :])
                gkh_c = gst.tile([128, GCHUNK, 32], F32R, tag="gkh")
                gkhb_c = gst.tile([128, GCHUNK, 32], BF16, tag="gkhb")
                gkl_c = gst.tile([128, GCHUNK, 32], F32R, tag="gkl")
                nc.sync.dma_start(out=gkh_c[:, :ce - cs, :], in_=gkh[:, cs:ce, :])
                nc.sync.dma_start(out=gkhb_c[:, :ce - cs, :], in_=gkhb[:, cs:ce, :])
                nc.sync.dma_start(out=gkl_c[:, :ce - cs, :], in_=gkl[:, cs:ce, :])
                for p in range(cs, ce):
                    pl = p - cs
                    rhs_h = xgc[:, 2 * pl, :]
                    rhs_l = xlc[:, 2 * pl, :]
                    nc.tensor.matmul(lgp[:], gkh_c[:, pl, :], rhs_h,
                                     start=first, stop=False)
                    first = False
                    imm += 1
                    nc.tensor.matmul(lgp[:], gkl_c[:, pl, :], rhs_h,
                                     start=False, stop=False)
                    imm += 1
                    nc.tensor.matmul(lgp[:], gkhb_c[:, pl, :], rhs_l,
                                     start=False, stop=(imm + 1 == nmm))
                    imm += 1

            # softmax over experts (transpose to [b, te] layout first)
            lgs = wk.tile([32, 64], F32)
            nc.scalar.copy(lgs[:], lgp[:])
            ptr1 = ps.tile([64, 64], F32, tag="mix")
            nc.tensor.transpose(ptr1[:, :32], lgs[:], eye_t[:32, :32])
            mx = wk.tile([64, 4], F32)
            for t in range(T):
                nc.vector.tensor_reduce(mx[:, t:t + 1], ptr1[:, 8 * t:8 * t + 8],
                                        axis=mybir.AxisListType.X,
                                        op=mybir.AluOpType.max)
            negmx = wk.tile([64, 4], F32)
            nc.gpsimd.tensor_scalar_mul(negmx[:], mx[:], -1.0)
            exps = wk.tile([64, 32], F32)
            sums = wk.tile([64, 4], F32)
            for t in range(T):
                nc.scalar.activation(exps[:, 8 * t:8 * t + 8], ptr1[:, 8 * t:8 * t + 8],
                                     mybir.ActivationFunctionType.Exp,
                                     bias=negmx[:, t:t + 1],
                                     accum_out=sums[:, t:t + 1])
            rsum = wk.tile([64, 4], F32)
            nc.vector.reciprocal(rsum[:], sums[:])
            gts = wk.tile([64, 32], F32)
            for t in range(T):
                nc.gpsimd.tensor_scalar_mul(gts[:, 8 * t:8 * t + 8],
                                            exps[:, 8 * t:8 * t + 8],
                                            rsum[:, t:t + 1])
            rg = wk.tile([8, 256], F32)
            for t in range(T):
                ptr2 = ps.tile([8, 64], F32, tag="mix", name=f"ptr2_{t}")
                nc.tensor.transpose(ptr2[:], gts[:, 8 * t:8 * t + 8], eye_t[:])
                nc.scalar.copy(rg[0:8, 64 * t:64 * t + 64], ptr2[:])
            pgc = ps.tile([128, 256], F32, tag="mix")
            nc.tensor.matmul(pgc[:], e8_t[:], rg[:], start=True, stop=True)
            gcs = wk.tile([128, 256], F32)
            nc.scalar.copy(gcs[:], pgc[:])
            g_tiles = []
            for gt in range(8):
                g_sub = wk.tile([128, 8 * 64], F32R, name=f"g_sub{gt}")
                for bl in range(8):
                    b = 8 * gt + bl
                    for t in range(T):
                        nc.gpsimd.tensor_scalar_mul(
                            g_sub[:, 64 * bl + 16 * t:64 * bl + 16 * t + 16],
                            d16_t[:], gcs[:, 64 * t + b:64 * t + b + 1])
                g_tiles.append(g_sub)

            # =========== conv + mixture phase ===========
            odv = out[:].rearrange("t s (c l) -> t s c l", c=C2)
            for sp in range(BC // 2):
                spair = (2 * sp, 2 * sp + 1)
                pc1s = {}
                h1all = {}
                for pr in range(4):
                    for s in spair:
                        pc1 = ps.tile([128, 400], F32, tag="c1", bufs=3,
                                      name=f"pc1_{s}_{pr}")
                        nc.tensor.matmul(pc1[:], w1a_t[:, pr, :],
                                         x2h_t[:, s, 0:400],
                                         start=True, stop=False)
                        pc1s[s] = pc1
                    for s in spair:
                        nc.tensor.matmul(pc1s[s][:], w1b_t[:, pr, :],
                                         x2h_t[:, s, 2:402],
                                         start=False, stop=True)
                    for s in spair:
                        h1t = cv.tile([128, 400], F32R, tag="h1", bufs=10,
                                      name=f"h1_{s}_{pr}")
                        nc.scalar.activation(h1t[:], pc1s[s][:],
                                             mybir.ActivationFunctionType.Relu,
                                             bias=b1_t[:, pr:pr + 1])
                        h1all[(s, pr)] = h1t
                eofs = {}
                for s in spair:
                    eofs[s] = cv.tile([128, 400], F32, tag="eof", bufs=2,
                                      name=f"eof_{s}")
                for pr in range(4):
                    pc2s = {}
                    for s in spair:
                        pc2s[s] = ps.tile([32, 400], F32, tag="c2", bufs=3,
                                          name=f"pc2_{s}_{pr}")
                    # taps in order 2,1,0: tap2 covers all columns with start=True
                    for k, ob, rb in ((2, 0, 0), (1, 2, 0), (0, 4, 0)):
                        for s in spair:
                            nc.tensor.matmul(pc2s[s][:, ob:400],
                                             w2_t[:, 3 * pr + k, :],
                                             h1all[(s, pr)][:, rb:400 - ob],
                                             start=(k == 2), stop=(k == 0))
                    for s in spair:
                        nc.vector.tensor_scalar(eofs[s][32 * pr:32 * pr + 32, :],
                                                pc2s[s][:],
                                                b2_t[:, pr:pr + 1], 0.0,
                                                mybir.AluOpType.add,
                                                mybir.AluOpType.max)
                for s in spair:
                    eof2 = cv.tile([128, 400], F32, tag="eof2", bufs=2,
                                   name=f"eof2_{s}")
                    nc.gpsimd.tensor_add(eof2[:], eofs[s][:], ebias_t[:])
                    eor = cv.tile([128, 400], F32R, tag="eor", bufs=2,
                                  name=f"eor_{s}")
                    nc.scalar.activation(eor[:], eof2[:],
                                         mybir.ActivationFunctionType.Relu)
                    pmx = ps.tile([64, 400], F32, tag="mix", name=f"pmx_{s}")
                    nc.tensor.matmul(pmx[:], g_tiles[s // 8][:, 64 * (s % 8):64 * (s % 8) + 64], eor[:],
                                     start=True, stop=True)
                    outs = cv.tile([64, 400], F32, tag="outs", bufs=3,
                                   name=f"outs_{s}")
                    nc.scalar.activation(outs[:], pmx[:],
                                         mybir.ActivationFunctionType.Identity,
                                         bias=tb_t[:])
                    for t in range(T):
                        nc.sync.dma_start(out=odv[t, s],
                                          in_=outs[16 * t:16 * t + 16, :])
    nc.compile()
    return nc


_NC_CACHE = None


def _get_nc():
    global _NC_CACHE
    if _NC_CACHE is None:
        _NC_CACHE = build_nc()
    return _NC_CACHE


def kernel(x, conv1_w, conv1_b, conv2_w, conv2_b, expert_bias,
           gate_kernels, gate_bias, task_bias):
    global LAST_EXEC_NS
    x = np.asarray(x, np.float32)
    conv1_w = np.asarray(conv1_w, np.float32)
    conv1_b = np.asarray(conv1_b, np.float32)
    conv2_w = np.asarray(conv2_w, np.float32)
    conv2_b = np.asarray(conv2_b, np.float32)
    expert_bias = np.asarray(expert_bias, np.float32)
    gate_kernels = np.asarray(gate_kernels, np.float32)
    gate_bias = np.asarray(gate_bias, np.float32)
    task_bias = np.asarray(task_bias, np.float32)

    # ---- host prep ----
    xh = f32r_round(x)                       # [B, F, L]
    xl = (x - xh).astype(np.float32)

    # conv1/gate input, tap-shifted duplicate: [128, B, 402]
    x2h_full = np.zeros((128, B, LP), np.float32)
    xh_t = xh.transpose(1, 0, 2)             # [F, B, L]
    x2h_full[:64, :, 2:402] = xh_t
    x2h_full[64:, :, 1:401] = xh_t
    # gate lo-part, l-major: [128, 402, B] bf16
    x2l_full = np.zeros((128, LP, B), np.float32)
    xl_t = xl.transpose(1, 2, 0)             # [F, L, B]
    x2l_full[:64, 2:402, :] = xl_t
    x2l_full[64:, 1:401, :] = xl_t
    x2g_full = np.zeros((128, LP, B), np.float32)
    xh_lt = xh.transpose(1, 2, 0)            # [F, L, B]
    x2g_full[:64, 2:402, :] = xh_lt
    x2g_full[64:, 1:401, :] = xh_lt
    x2l_full = x2l_full.astype(ml_dtypes.bfloat16)

    # gate kernels: [T, F*L, E] -> lhsT chunks [128=(j,f), 200, 32=(t,e)]
    gk = gate_kernels.reshape(T, F, L, E)
    gkh_v = f32r_round(gk)
    gkl_v = f32r_round(gk - gkh_v)

    def gk_pack(g):
        # out[(j*64+f), p, t*8+e] = g[t, f, 2p+j, e]
        gp = g.reshape(T, F, NLPAIR, 2, E)        # t f p j e
        gp = gp.transpose(3, 1, 2, 0, 4)          # j f p t e
        return np.ascontiguousarray(gp.reshape(128, NLPAIR, 32))

    gkh_a = gk_pack(gkh_v)
    gkl_a = gk_pack(gkl_v)
    gkhb_a = gkh_a.astype(ml_dtypes.bfloat16)

    # conv1 weights: pair pr -> lhsT [128=(j,f), 128=(i,c1)]
    w1r = f32r_round(conv1_w)                # [E, C1, F, K]
    w1a_a = np.zeros((4, 128, 128), np.float32)
    w1b_a = np.zeros((4, 128, 128), np.float32)
    for pr in range(4):
        for i in range(2):
            e = 2 * pr + i
            for j in range(2):
                w1a_a[pr, 64 * j:64 * j + 64, 64 * i:64 * i + 64] = w1r[e, :, :, j].T
            w1b_a[pr, 0:64, 64 * i:64 * i + 64] = w1r[e, :, :, 2].T
    b1_a = np.zeros((128, 4), np.float32)
    for pr in range(4):
        for i in range(2):
            b1_a[64 * i:64 * i + 64, pr] = conv1_b[2 * pr + i]

    # conv2 weights: [4, 3, 128=(i,c1), 32=(i,c2)] block-diag per pair
    w2r = f32r_round(conv2_w)                # [E, C2, C1, K]
    w2_a = np.zeros((4, 3, 128, 32), np.float32)
    for pr in range(4):
        for i in range(2):
            e = 2 * pr + i
            for k in range(3):
                w2_a[pr, k, 64 * i:64 * i + 64, 16 * i:16 * i + 16] = w2r[e, :, :, k].T

    # expert bias (+conv2 bias) dense [128=(e,c2), 400]
    ebias_a = expert_bias.reshape(128, 400).astype(np.float32)
    b2_a = np.zeros((32, 4), np.float32)
    for pr in range(4):
        for i in range(2):
            b2_a[16 * i:16 * i + 16, pr] = conv2_b[2 * pr + i]

    d16_a = np.tile(np.eye(16, dtype=np.float32), (8, 1))          # [128, 16]
    e8_a = np.zeros((8, 128), np.float32)
    for e in range(8):
        e8_a[e, 16 * e:16 * e + 16] = 1.0
    tb_a = np.repeat(task_bias, 16).reshape(64, 1).astype(np.float32)
    eye_a = np.eye(64, dtype=np.float32)
    gbb1_a = gate_bias.reshape(1, 32).astype(np.float32)
    ones1_a = np.ones((1, 64), np.float32)

    w1a_a = np.ascontiguousarray(w1a_a.transpose(1, 0, 2))      # [128, 4, 128]
    w1b_a = np.ascontiguousarray(w1b_a.transpose(1, 0, 2))
    w2_a = np.ascontiguousarray(w2_a.transpose(2, 0, 1, 3).reshape(128, 12, 32))
    nc = _get_nc()
    shared = {"gkh": gkh_a, "gkhb": gkhb_a, "gkl": gkl_a,
              "w1a": w1a_a, "w1b": w1b_a, "w2": w2_a, "b1": b1_a,
              "ebias": ebias_a, "b2": b2_a, "d16": d16_a, "e8": e8_a, "tb": tb_a,
              "eye": eye_a, "gbb1": gbb1_a, "ones1": ones1_a}
    in_maps = []
    for c in range(NCORES):
        m = dict(shared)
        m["x2h"] = np.ascontiguousarray(x2h_full[:, BC * c:BC * (c + 1), :])
        m["x2l"] = np.ascontiguousarray(x2l_full[:, :, BC * c:BC * (c + 1)])
        m["x2g"] = np.ascontiguousarray(x2g_full[:, :, BC * c:BC * (c + 1)])
        in_maps.append(m)

    trace = bool(os.environ.get("BASS_KERNEL_TRACE"))
    res = run_bass_kernel_spmd(nc, in_maps, core_ids=list(range(NCORES)),
                               trace=trace)
    LAST_EXEC_NS = res.exec_time_ns
    out = np.empty((T, B, U), np.float32)
    for c in range(NCORES):
        out[:, BC * c:BC * (c + 1), :] = res.results[c]["out"]
    return out


# revision 13
# speedup vs baseline: 1.2942x; 1.2942x over previous
"""MMoE (8-expert TCN mixture) Trainium2 kernel.

Data-parallel over batch: 8 cores x 64 samples, all params replicated.
All matmuls in float32r (11-bit RNE mantissa, ~full PE rate at K=128) with a
hi/lo-split gate matmul (softmax over 25600-term logits needs ~fp32 accuracy).

Self-contained: hardcodes shapes, preprocesses inputs on host (padding,
shifted-duplicate layouts, f32r pre-rounding), builds one Bass/Tile program,
runs it SPMD on 8 NeuronCores, reassembles the full [4, 512, 6400] output.
"""
import os
import sys

sys.path.insert(0, "/opt/trn_rl_repo")

import numpy as np
import ml_dtypes

import concourse.bass as bass
import concourse.tile as tile
from concourse import bacc, mybir
from concourse.bass_utils import run_bass_kernel_spmd

F32 = mybir.dt.float32
F32R = mybir.dt.float32r
BF16 = mybir.dt.bfloat16

B, F, L = 512, 64, 400
E, T = 8, 4
C1, C2, KW = 64, 16, 3
U = C2 * L
NCORES = 8
BC = B // NCORES          # samples per core
LP = 402                  # padded seq len for conv1 input (left pad 2)
HP = 404                  # padded seq len for conv2 input (left pad 4)
NLPAIR = L // 2           # 200 gate l-pair chunks
GCHUNK = 20               # l-pairs per gate stream chunk

LAST_EXEC_NS = None       # set after each kernel() call when tracing


def f32r_round(a: np.ndarray) -> np.ndarray:
    """Round fp32 to the float32r grid: RNE to 11 explicit mantissa bits."""
    u = np.ascontiguousarray(a, dtype=np.float32).view(np.uint32)
    s = u & np.uint32(0x80000000)
    m = u & np.uint32(0x7FFFFFFF)
    lsb = (m >> np.uint32(12)) & np.uint32(1)
    m2 = (m + np.uint32(0x7FF) + lsb) & np.uint32(0xFFFFF000)
    return (s | m2).view(np.float32)


def build_nc():
    nc = bacc.Bacc("TRN2", target_bir_lowering=False)

    x2h = nc.dram_tensor("x2h", [128, BC, LP], F32R, kind="ExternalInput")
    x2l = nc.dram_tensor("x2l", [128, LP, BC], BF16, kind="ExternalInput")
    x2g = nc.dram_tensor("x2g", [128, LP, BC], F32R, kind="ExternalInput")
    gkh = nc.dram_tensor("gkh", [128, NLPAIR, 32], F32R, kind="ExternalInput")
    gkhb = nc.dram_tensor("gkhb", [128, NLPAIR, 32], BF16, kind="ExternalInput")
    gkl = nc.dram_tensor("gkl", [128, NLPAIR, 32], F32R, kind="ExternalInput")
    w1a = nc.dram_tensor("w1a", [128, 4, 128], F32R, kind="ExternalInput")
    w1b = nc.dram_tensor("w1b", [128, 4, 128], F32R, kind="ExternalInput")
    w2 = nc.dram_tensor("w2", [128, 12, 32], F32R, kind="ExternalInput")
    b1 = nc.dram_tensor("b1", [128, 4], F32, kind="ExternalInput")
    ebias = nc.dram_tensor("ebias", [128, 400], F32, kind="ExternalInput")
    b2 = nc.dram_tensor("b2", [32, 4], F32, kind="ExternalInput")
    d16 = nc.dram_tensor("d16", [128, 16], F32, kind="ExternalInput")
    e8 = nc.dram_tensor("e8", [8, 128], F32, kind="ExternalInput")
    tb = nc.dram_tensor("tb", [64, 1], F32, kind="ExternalInput")
    eye = nc.dram_tensor("eye", [64, 64], F32, kind="ExternalInput")
    gbb1 = nc.dram_tensor("gbb1", [1, 32], F32, kind="ExternalInput")
    ones1 = nc.dram_tensor("ones1", [1, 64], F32, kind="ExternalInput")
    out = nc.dram_tensor("out", [T, BC, U], F32, kind="ExternalOutput")

    with tile.TileContext(nc) as tc:
        with (
            tc.tile_pool(name="const", bufs=1) as cst,
            tc.tile_pool(name="xres", bufs=1) as xres,
            tc.tile_pool(name="gstream", bufs=2) as gst,
            tc.tile_pool(name="work", bufs=1) as wk,
            tc.tile_pool(name="conv", bufs=1) as cv,
            tc.tile_pool(name="ps", bufs=1, space="PSUM") as ps,
        ):
            # ---- constants ----
            w1a_t = cst.tile([128, 4, 128], F32R)
            w1b_t = cst.tile([128, 4, 128], F32R)
            w2_t = cst.tile([128, 12, 32], F32R)
            b1_t = cst.tile([128, 4], F32)
            ebias_t = cst.tile([128, 400], F32)
            b2_t = cst.tile([32, 4], F32)
            d16_t = cst.tile([128, 16], F32)
            e8_t = cst.tile([8, 128], F32)
            tb_t = cst.tile([64, 1], F32)
            eye_t = cst.tile([64, 64], F32)
            gbb1_t = cst.tile([1, 32], F32)
            ones1_t = cst.tile([1, 64], F32)
            for tl, dr in ((w1a_t, w1a), (w1b_t, w1b), (w2_t, w2), (b1_t, b1),
                           (ebias_t, ebias), (b2_t, b2), (d16_t, d16), (e8_t, e8),
                           (tb_t, tb), (eye_t, eye), (gbb1_t, gbb1), (ones1_t, ones1)):
                nc.sync.dma_start(out=tl[:], in_=dr[:])

            # ---- resident x (f32r, tap-shifted duplicate layout) ----
            x2h_t = xres.tile([128, BC, LP], F32R)
            nc.gpsimd.dma_start(out=x2h_t[:], in_=x2h[:])

            # =========== gates phase ===========
            lgp = ps.tile([32, 64], F32, tag="lg")
            nc.tensor.matmul(lgp[:], gbb1_t[:], ones1_t[:], start=True, stop=False)
            first = False
            nmm = 3 * NLPAIR
            imm = 0
            for cs in range(0, NLPAIR, GCHUNK):
                ce = min(cs + GCHUNK, NLPAIR)
                ncols = 2 * (ce - cs)
                xlc = gst.tile([128, 2 * GCHUNK, BC], BF16, tag="xlc")
                nc.sync.dma_start(out=xlc[:, :ncols, :], in_=x2l[:, 2 * cs + 2:2 * ce + 2, :])
                use_xg = True
                if use_xg:
                    xgc = gst.tile([128, 2 * GCHUNK, BC], F32R, tag="xgc",
                                   name=f"xgc_{cs}")
                    nc.sync.dma_start(out=xgc[:, :ncols, :],
                                      in_=x2g[:, 2 * cs + 2:2 * ce + 2, :])
                gkh_c = gst.tile([128, GCHUNK, 32], F32R, tag="gkh")
                gkhb_c = gst.tile([128, GCHUNK, 32], BF16, tag="gkhb")
                gkl_c = gst.tile([128, GCHUNK, 32], F32R, tag="gkl")
                nc.sync.dma_start(out=gkh_c[:, :ce - cs, :], in_=gkh[:, cs:ce, :])
                nc.sync.dma_start(out=gkhb_c[:, :ce - cs, :], in_=gkhb[:, cs:ce, :])
                nc.sync.dma_start(out=gkl_c[:, :ce - cs, :], in_=gkl[:, cs:ce, :])
                for p in range(cs, ce):
                    pl = p - cs
                    rhs_h = xgc[:, 2 * pl, :]
                    rhs_l = xlc[:, 2 * pl, :]
                    nc.tensor.matmul(lgp[:], gkh_c[:, pl, :], rhs_h,
                                     start=first, stop=False)
                    first = False
                    imm += 1
                    nc.tensor.matmul(lgp[:], gkl_c[:, pl, :], rhs_h,
                                     start=False, stop=False)
                    imm += 1
                    nc.tensor.matmul(lgp[:], gkhb_c[:, pl, :], rhs_l,
                                     start=False, stop=(imm + 1 == nmm))
                    imm += 1

            # softmax over experts (transpose to [b, te] layout first)
            lgs = wk.tile([32, 64], F32)
            nc.scalar.copy(lgs[:], lgp[:])
            ptr1 = ps.tile([64, 64], F32, tag="mix")
            nc.tensor.transpose(ptr1[:, :32], lgs[:], eye_t[:32, :32])
            mx = wk.tile([64, 4], F32)
            for t in range(T):
                nc.vector.tensor_reduce(mx[:, t:t + 1], ptr1[:, 8 * t:8 * t + 8],
                                        axis=mybir.AxisListType.X,
                                        op=mybir.AluOpType.max)
            negmx = wk.tile([64, 4], F32)
            nc.gpsimd.tensor_scalar_mul(negmx[:], mx[:], -1.0)
            exps = wk.tile([64, 32], F32)
            sums = wk.tile([64, 4], F32)
            for t in range(T):
                nc.scalar.activation(exps[:, 8 * t:8 * t + 8], ptr1[:, 8 * t:8 * t + 8],
                                     mybir.ActivationFunctionType.Exp,
                                     bias=negmx[:, t:t + 1],
                                     accum_out=sums[:, t:t + 1])
            rsum = wk.tile([64, 4], F32)
            nc.vector.reciprocal(rsum[:], sums[:])
            gts = wk.tile([64, 32], F32)
            for t in range(T):
                nc.gpsimd.tensor_scalar_mul(gts[:, 8 * t:8 * t + 8],
                                            exps[:, 8 * t:8 * t + 8],
                                            rsum[:, t:t + 1])
            rg = wk.tile([8, 256], F32)
            for t in range(T):
                ptr2 = ps.tile([8, 64], F32, tag="mix", name=f"ptr2_{t}")
                nc.tensor.transpose(ptr2[:], gts[:, 8 * t:8 * t + 8], eye_t[:])
                nc.scalar.copy(rg[0:8, 64 * t:64 * t + 64], ptr2[:])
            pgc = ps.tile([128, 256], F32, tag="mix")
            nc.tensor.matmul(pgc[:], e8_t[:], rg[:], start=True, stop=True)
            gcs = wk.tile([128, 256], F32)
            nc.scalar.copy(gcs[:], pgc[:])
            g_tiles = []
            for gt in range(8):
                g_sub = wk.tile([128, 8 * 64], F32R, name=f"g_sub{gt}")
                for bl in range(8):
                    b = 8 * gt + bl
                    for t in range(T):
                        nc.gpsimd.tensor_scalar_mul(
                            g_sub[:, 64 * bl + 16 * t:64 * bl + 16 * t + 16],
                            d16_t[:], gcs[:, 64 * t + b:64 * t + b + 1])
                g_tiles.append(g_sub)

            # =========== conv + mixture phase ===========
            odv = out[:].rearrange("t s (c l) -> t s c l", c=C2)
            for sp in range(BC // 2):
                spair = (2 * sp, 2 * sp + 1)
                pc1s = {}
                h1all = {}
                for pr in range(4):
                    for s in spair:
                        pc1 = ps.tile([128, 400], F32, tag="c1", bufs=3,
                                      name=f"pc1_{s}_{pr}")
                        nc.tensor.matmul(pc1[:], w1a_t[:, pr, :],
                                         x2h_t[:, s, 0:400],
                                         start=True, stop=False)
                        pc1s[s] = pc1
                    for s in spair:
                        nc.tensor.matmul(pc1s[s][:], w1b_t[:, pr, :],
                                         x2h_t[:, s, 2:402],
                                         start=False, stop=True)
                    for s in spair:
                        h1t = cv.tile([128, 400], F32R, tag="h1", bufs=10,
                                      name=f"h1_{s}_{pr}")
                        nc.scalar.activation(h1t[:], pc1s[s][:],
                                             mybir.ActivationFunctionType.Relu,
                                             bias=b1_t[:, pr:pr + 1])
                        h1all[(s, pr)] = h1t
                eofs = {}
                for s in spair:
                    eofs[s] = cv.tile([128, 400], F32, tag="eof", bufs=2,
                                      name=f"eof_{s}")
                for pr in range(4):
                    pc2s = {}
                    for s in spair:
                        pc2s[s] = ps.tile([32, 400], F32, tag="c2", bufs=3,
                                          name=f"pc2_{s}_{pr}")
                    # taps in order 2,1,0: tap2 covers all columns with start=True
                    for k, ob, rb in ((2, 0, 0), (1, 2, 0), (0, 4, 0)):
                        for s in spair:
                            nc.tensor.matmul(pc2s[s][:, ob:400],
                                             w2_t[:, 3 * pr + k, :],
                                             h1all[(s, pr)][:, rb:400 - ob],
                                             start=(k == 2), stop=(k == 0))
                    for s in spair:
                        nc.vector.tensor_scalar(eofs[s][32 * pr:32 * pr + 32, :],
                                                pc2s[s][:],
                                                b2_t[:, pr:pr + 1], 0.0,
                                                mybir.AluOpType.add,
                                                mybir.AluOpType.max)
                for s in spair:
                    eof2 = cv.tile([128, 400], F32, tag="eof2", bufs=2,
                                   name=f"eof2_{s}")
                    nc.vector.tensor_add(eof2[:], eofs[s][:], ebias_t[:])
                    eor = cv.tile([128, 400], F32R, tag="eor", bufs=2,
                                  name=f"eor_{s}")
                    nc.scalar.activation(eor[:], eof2[:],
                                         mybir.ActivationFunctionType.Relu)
                    pmx = ps.tile([64, 400], F32, tag="mix", name=f"pmx_{s}")
                    nc.tensor.matmul(pmx[:], g_tiles[s // 8][:, 64 * (s % 8):64 * (s % 8) + 64], eor[:],
                                     start=True, stop=True)
                    outs = cv.tile([64, 400], F32, tag="outs", bufs=3,
                                   name=f"outs_{s}")
                    nc.scalar.activation(outs[:], pmx[:],
                                         mybir.ActivationFunctionType.Identity,
                                         bias=tb_t[:])
                    for t in range(T):
                        nc.sync.dma_start(out=odv[t, s],
                                          in_=outs[16 * t:16 * t + 16, :])
    nc.compile()
    return nc


_NC_CACHE = None


def _get_nc():
    global _NC_CACHE
    if _NC_CACHE is None:
        _NC_CACHE = build_nc()
    return _NC_CACHE


def kernel(x, conv1_w, conv1_b, conv2_w, conv2_b, expert_bias,
           gate_kernels, gate_bias, task_bias):
    global LAST_EXEC_NS
    x = np.asarray(x, np.float32)
    conv1_w = np.asarray(conv1_w, np.float32)
    conv1_b = np.asarray(conv1_b, np.float32)
    conv2_w = np.asarray(conv2_w, np.float32)
    conv2_b = np.asarray(conv2_b, np.float32)
    expert_bias = np.asarray(expert_bias, np.float32)
    gate_kernels = np.asarray(gate_kernels, np.float32)
    gate_bias = np.asarray(gate_bias, np.float32)
    task_bias = np.asarray(task_bias, np.float32)

    # ---- host prep ----
    xh = f32r_round(x)                       # [B, F, L]
    xl = (x - xh).astype(np.float32)

    # conv1/gate input, tap-shifted duplicate: [128, B, 402]
    x2h_full = np.zeros((128, B, LP), np.float32)
    xh_t = xh.transpose(1, 0, 2)             # [F, B, L]
    x2h_full[:64, :, 2:402] = xh_t
    x2h_full[64:, :, 1:401] = xh_t
    # gate lo-part, l-major: [128, 402, B] bf16
    x2l_full = np.zeros((128, LP, B), np.float32)
    xl_t = xl.transpose(1, 2, 0)             # [F, L, B]
    x2l_full[:64, 2:402, :] = xl_t
    x2l_full[64:, 1:401, :] = xl_t
    x2g_full = np.zeros((128, LP, B), np.float32)
    xh_lt = xh.transpose(1, 2, 0)            # [F, L, B]
    x2g_full[:64, 2:402, :] = xh_lt
    x2g_full[64:, 1:401, :] = xh_lt
    x2l_full = x2l_full.astype(ml_dtypes.bfloat16)

    # gate kernels: [T, F*L, E] -> lhsT chunks [128=(j,f), 200, 32=(t,e)]
    gk = gate_kernels.reshape(T, F, L, E)
    gkh_v = f32r_round(gk)
    gkl_v = f32r_round(gk - gkh_v)

    def gk_pack(g):
        # out[(j*64+f), p, t*8+e] = g[t, f, 2p+j, e]
        gp = g.reshape(T, F, NLPAIR, 2, E)        # t f p j e
        gp = gp.transpose(3, 1, 2, 0, 4)          # j f p t e
        return np.ascontiguousarray(gp.reshape(128, NLPAIR, 32))

    gkh_a = gk_pack(gkh_v)
    gkl_a = gk_pack(gkl_v)
    gkhb_a = gkh_a.astype(ml_dtypes.bfloat16)

    # conv1 weights: pair pr -> lhsT [128=(j,f), 128=(i,c1)]
    w1r = f32r_round(conv1_w)                # [E, C1, F, K]
    w1a_a = np.zeros((4, 128, 128), np.float32)
    w1b_a = np.zeros((4, 128, 128), np.float32)
    for pr in range(4):
        for i in range(2):
            e = 2 * pr + i
            for j in range(2):
                w1a_a[pr, 64 * j:64 * j + 64, 64 * i:64 * i + 64] = w1r[e, :, :, j].T
            w1b_a[pr, 0:64, 64 * i:64 * i + 64] = w1r[e, :, :, 2].T
    b1_a = np.zeros((128, 4), np.float32)
    for pr in range(4):
        for i in range(2):
            b1_a[64 * i:64 * i + 64, pr] = conv1_b[2 * pr + i]

    # conv2 weights: [4, 3, 128=(i,c1), 32=(i,c2)] block-diag per pair
    w2r = f32r_round(conv2_w)                # [E, C2, C1, K]
    w2_a = np.zeros((4, 3, 128, 32), np.float32)
    for pr in range(4):
        for i in range(2):
            e = 2 * pr + i
            for k in range(3):
                w2_a[pr, k, 64 * i:64 * i + 64, 16 * i:16 * i + 16] = w2r[e, :, :, k].T

    # expert bias (+conv2 bias) dense [128=(e,c2), 400]
    ebias_a = expert_bias.reshape(128, 400).astype(np.float32)
    b2_a = np.zeros((32, 4), np.float32)
    for pr in range(4):
        for i in range(2):
            b2_a[16 * i:16 * i + 16, pr] = conv2_b[2 * pr + i]

    d16_a = np.tile(np.eye(16, dtype=np.float32), (8, 1))          # [128, 16]
    e8_a = np.zeros((8, 128), np.float32)
    for e in range(8):
        e8_a[e, 16 * e:16 * e + 16] = 1.0
    tb_a = np.repeat(task_bias, 16).reshape(64, 1).astype(np.float32)
    eye_a = np.eye(64, dtype=np.float32)
    gbb1_a = gate_bias.reshape(1, 32).astype(np.float32)
    ones1_a = np.ones((1, 64), np.float32)

    w1a_a = np.ascontiguousarray(w1a_a.transpose(1, 0, 2))      # [128, 4, 128]
    w1b_a = np.ascontiguousarray(w1b_a.transpose(1, 0, 2))
    w2_a = np.ascontiguousarray(w2_a.transpose(2, 0, 1, 3).reshape(128, 12, 32))
    nc = _get_nc()
    shared = {"gkh": gkh_a, "gkhb": gkhb_a, "gkl": gkl_a,
              "w1a": w1a_a, "w1b": w1b_a, "w2": w2_a, "b1": b1_a,
              "ebias": ebias_a, "b2": b2_a, "d16": d16_a, "e8": e8_a, "tb": tb_a,
              "eye": eye_a, "gbb1": gbb1_a, "ones1": ones1_a}
    in_maps = []
    for c in range(NCORES):
        m = dict(shared)
        m["x2h"] = np.ascontiguousarray(x2h_full[:, BC * c:BC * (c + 1), :])
        m["x2l"] = np.ascontiguousarray(x2l_full[:, :, BC * c:BC * (c + 1)])
        m["x2g"] = np.ascontiguousarray(x2g_full[:, :, BC * c:BC * (c + 1)])
        in_maps.append(m)

    trace = bool(os.environ.get("BASS_KERNEL_TRACE"))
    res = run_bass_kernel_spmd(nc, in_maps, core_ids=list(range(NCORES)),
                               trace=trace)
    LAST_EXEC_NS = res.exec_time_ns
    out = np.empty((T, B, U), np.float32)
    for c in range(NCORES):
        out[:, BC * c:BC * (c + 1), :] = res.results[c]["out"]
    return out
